# revision 3
# baseline (speedup 1.0000x reference)
"""Trainium2 Bass kernel for nn_CompositionalMlp (4-node compositional MLP,
4 experts/node, exact one-hot routing), data-parallel over batch on 8 cores.

Layout: activations kept as [features, batch] ([D,B]) so weights are the
stationary matmul operand and no transposes are needed. All experts are
computed densely; per-row expert selection is done with PE-built broadcast
masks + DVE predicated copies. Matmuls run in float16 (1 cyc/row).
"""
import os
import sys
sys.path.insert(0, "/opt/trn_rl_repo")
os.environ.setdefault("NEURON_RT_RESET_CORES", "1")
import numpy as np

B = 65536
E = 4
F = 32
H = 256
NODES = 4
D_MID = 128
D_OUT = 8
N_CORES = 8
BC = B // N_CORES      # 8192 rows per core
CH = 512               # batch columns per chunk
NCH = BC // CH         # 16 chunks

_COMPILED = {}


def _build(f32r_io: bool):
    import concourse.bass as bass  # noqa: F401
    from concourse import bacc
    import concourse.mybir as mybir
    from concourse.tile import TileContext

    F32 = mybir.dt.float32
    F32R = mybir.dt.float16
    I32 = mybir.dt.int32
    ADD = mybir.AluOpType.add
    MAX = mybir.AluOpType.max
    DT_IN = F32R

    nc = bacc.Bacc("TRN2", target_bir_lowering=False, debug=False,
                   num_devices=N_CORES)
    xT = nc.dram_tensor("xT", [144, BC], DT_IN, kind="ExternalInput").ap()
    win = nc.dram_tensor("win", [16, 32, 256], DT_IN, kind="ExternalInput").ap()
    wa = nc.dram_tensor("wa", [12, 128, 256], DT_IN, kind="ExternalInput").ap()
    wb = nc.dram_tensor("wb", [12, 256, 256], DT_IN, kind="ExternalInput").ap()
    wu = nc.dram_tensor("wu", [12, 256, 128], DT_IN, kind="ExternalInput").ap()
    w3u = nc.dram_tensor("w3u", [4, 256, 8], DT_IN, kind="ExternalInput").ap()
    b3u = nc.dram_tensor("b3u", [4, 8], DT_IN, kind="ExternalInput").ap()
    mpat = nc.dram_tensor("mpat", [3, 4, 128], DT_IN, kind="ExternalInput").ap()
    binb = nc.dram_tensor("binb", [128, 32], F32, kind="ExternalInput").ap()
    bhb = nc.dram_tensor("bhb", [128, 24], F32, kind="ExternalInput").ap()
    cbb = nc.dram_tensor("cbb", [128, 3], F32, kind="ExternalInput").ap()
    yT = nc.dram_tensor("yT", [8, BC], F32, kind="ExternalOutput").ap()

    with TileContext(nc) as tc:
        with (
            tc.tile_pool(name="wload", bufs=1) as wl,
            tc.tile_pool(name="wpool", bufs=1) as wp,
            tc.tile_pool(name="io", bufs=3) as io,
            tc.tile_pool(name="acts", bufs=2) as ap_,
            tc.tile_pool(name="psh", bufs=3, space="PSUM") as psh,
            tc.tile_pool(name="psu", bufs=2, space="PSUM") as psu,
            tc.tile_pool(name="psm", bufs=3, space="PSUM") as psm,
        ):
            def wtile(shape, tag, src):
                t = wp.tile(shape, F32R, tag=tag)
                nc.sync.dma_start(t[:, :], src)
                return t

            # --- resident weights ---
            win_t = [wtile([32, 256], f"win{i}", win[i]) for i in range(16)]
            wa_t = [wtile([128, 256], f"wa{i}", wa[i]) for i in range(12)]
            wb_t = [[wtile([128, 256], f"wb{i}_{k}", wb[i][k * 128:(k + 1) * 128, :])
                     for k in range(2)] for i in range(12)]
            wu_t = [[wtile([128, 128], f"wu{i}_{k}", wu[i][k * 128:(k + 1) * 128, :])
                     for k in range(2)] for i in range(12)]
            w3u_t = [[wtile([128, 8], f"w3u{e}_{k}", w3u[e][k * 128:(k + 1) * 128, :])
                      for k in range(2)] for e in range(4)]
            b3u_t = wtile([4, 8], "b3u", b3u[:, :])
            mp_t = [wtile([4, 128], f"mp{i}", mpat[i]) for i in range(3)]
            bin_t = wp.tile([128, 32], F32, tag="binb")
            nc.sync.dma_start(bin_t[:, :], binb[:, :])
            bh_t = wp.tile([128, 24], F32, tag="bhb")
            nc.sync.dma_start(bh_t[:, :], bhb[:, :])
            cb_t = wp.tile([128, 3], F32, tag="cbb")
            nc.sync.dma_start(cb_t[:, :], cbb[:, :])

            for ci in range(NCH):
                S = slice(ci * CH, (ci + 1) * CH)
                # inputs: per-node feature rows + one-hot rows (base partition 0 each)
                xf = []
                oh = []
                for j in range(NODES):
                    t = io.tile([32, CH], DT_IN, tag=f"xf{j}")
                    nc.sync.dma_start(t[:, :], xT[32 * j:32 * (j + 1), S])
                    xf.append(t)
                    t = io.tile([4, CH], DT_IN, tag=f"oh{j}")
                    nc.sync.dma_start(t[:, :], xT[128 + 4 * j:132 + 4 * j, S])
                    oh.append(t)

                x_prev = None
                for j in range(NODES):
                    # input layer (node0: first layer; nodes 1-3: pre-interface)
                    hin = []
                    for m in range(8):
                        e = m // 2
                        ph = psh.tile([128, CH], F32, tag="ph")
                        nc.tensor.matmul(ph[:, :],
                                         win_t[4 * j + e][:, (m % 2) * 128:(m % 2 + 1) * 128],
                                         xf[j][:, :], start=True, stop=True)
                        ht = ap_.tile([128, CH], F32R, tag=f"a{m}")
                        nc.any.tensor_scalar(ht[:, :], ph[:, :],
                                             bin_t[:, 8 * j + m:8 * j + m + 1], 0.0, ADD, MAX)
                        hin.append(ht)

                    if j == 0:
                        hmid = hin          # node0 has no interface layer
                    else:
                        hmid = []
                        for m in range(8):
                            e = m // 2
                            k2 = m % 2
                            i12 = 4 * (j - 1) + e
                            ph = psh.tile([128, CH], F32, tag="ph")
                            nc.tensor.matmul(ph[:, :],
                                             wa_t[i12][:, k2 * 128:(k2 + 1) * 128],
                                             x_prev[:, :], start=True, stop=False)
                            nc.tensor.matmul(ph[:, :],
                                             wb_t[i12][0][:, k2 * 128:(k2 + 1) * 128],
                                             hin[2 * e][:, :], start=False, stop=False)
                            nc.tensor.matmul(ph[:, :],
                                             wb_t[i12][1][:, k2 * 128:(k2 + 1) * 128],
                                             hin[2 * e + 1][:, :], start=False, stop=True)
                            ht = ap_.tile([128, CH], F32R, tag=f"h{m}")
                            nc.any.tensor_scalar(ht[:, :], ph[:, :],
                                                 bh_t[:, 8 * (j - 1) + m:8 * (j - 1) + m + 1],
                                                 0.0, ADD, MAX)
                            hmid.append(ht)

                    # masks for this node's combine (experts 1..3)
                    pmasks = []
                    for e in (1, 2, 3):
                        mdim = 128 if j < 3 else 8
                        pm = psm.tile([128, CH], F32, tag="mask")
                        nc.tensor.matmul(pm[0:mdim, :], mp_t[e - 1][:, 0:mdim],
                                         oh[j][:, :], start=True, stop=True)
                        pmasks.append(pm)

                    if j < 3:
                        # combine: U_e = W1[e].T @ h_e (+0.1 relu), select expert
                        us = []
                        for e in range(4):
                            pu = psu.tile([128, CH], F32, tag="pu")
                            nc.tensor.matmul(pu[:, :], wu_t[4 * j + e][0][:, :],
                                             hmid[2 * e][:, :], start=True, stop=False)
                            nc.tensor.matmul(pu[:, :], wu_t[4 * j + e][1][:, :],
                                             hmid[2 * e + 1][:, :], start=False, stop=True)
                            ut = ap_.tile([128, CH], F32R, tag=f"u{e}")
                            nc.any.tensor_scalar(ut[:, :], pu[:, :],
                                                 cb_t[:, j:j + 1], 0.0, ADD, MAX)
                            us.append(ut)
                        for e in (1, 2, 3):
                            nc.vector.copy_predicated(us[0][:, :],
                                                      pmasks[e - 1][:, :].bitcast(I32),
                                                      us[e][:, :])
                        x_prev = us[0]
                    else:
                        # output head: U_e = W31[e].T @ h_e + sum_e oh_e b31[e]
                        ys = []
                        for e in range(4):
                            pu = psu.tile([8, CH], F32, tag="pu")
                            nc.tensor.matmul(pu[:, :], w3u_t[e][0][:, :],
                                             hmid[2 * e][:, :], start=True, stop=False)
                            nc.tensor.matmul(pu[:, :], w3u_t[e][1][:, :],
                                             hmid[2 * e + 1][:, :], start=False, stop=False)
                            nc.tensor.matmul(pu[:, :], b3u_t[:, :], oh[3][:, :],
                                             start=False, stop=True)
                            yt_ = ap_.tile([8, CH], F32, tag=f"y{e}")
                            nc.any.tensor_scalar(yt_[:, :], pu[:, :], 0.0, None, ADD)
                            ys.append(yt_)
                        for e in (1, 2, 3):
                            nc.vector.copy_predicated(ys[0][:, :],
                                                      pmasks[e - 1][0:8, :].bitcast(I32),
                                                      ys[e][:, :])
                        nc.sync.dma_start(yT[:, S], ys[0][:, :])
    nc.compile()
    return nc


def _prep_inputs(p):
    f32 = np.float32
    f16 = np.float16
    xT = np.ascontiguousarray(p["input_val"].T.astype(f16))          # [144, B]

    win = np.empty((16, 32, 256), f32)
    bin_ = np.empty((128, 32), f32)
    in_w = [p["W0_0"], p["W1_pre"], p["W2_pre"], p["W3_pre"]]
    in_b = [p["b0_0"], p["b1_pre"], p["b2_pre"], p["b3_pre"]]
    for j in range(4):
        for e in range(4):
            win[4 * j + e] = in_w[j][e]
            for half in range(2):
                bin_[:, 8 * j + 2 * e + half] = in_b[j][e][128 * half:128 * (half + 1)]

    wa = np.empty((12, 128, 256), f32)
    wb = np.empty((12, 256, 256), f32)
    bh = np.empty((128, 24), f32)
    for j in (1, 2, 3):
        w0 = p[f"W{j}_0"]
        b0 = p[f"b{j}_0"]
        for e in range(4):
            i12 = 4 * (j - 1) + e
            wa[i12] = w0[e][0:128, :]
            wb[i12] = w0[e][128:384, :]
            for half in range(2):
                bh[:, 8 * (j - 1) + 2 * e + half] = b0[e][128 * half:128 * (half + 1)]

    wu = np.empty((12, 256, 128), f32)
    cb = np.empty((128, 3), f32)
    for j in (0, 1, 2):
        w1 = p[f"W{j}_1"]
        b1 = p[f"b{j}_1"]
        assert np.ptp(b1, axis=0).max() == 0.0, "combine bias must be expert-constant"
        cb[:, j] = b1[0]
        for e in range(4):
            wu[4 * j + e] = w1[e]

    w3u = np.ascontiguousarray(p["W3_1"].astype(f32))                # [4, 256, 8]
    b3u = np.ascontiguousarray(p["b3_1"].astype(f32))                # [4, 8]
    w3u = w3u  # cast to fp16 in shared dict

    mpat = np.zeros((3, 4, 128), f32)
    for i, e in enumerate((1, 2, 3)):
        mpat[i, e, :] = 1.0

    shared = dict(win=win.astype(f16), wa=wa.astype(f16), wb=wb.astype(f16),
                  wu=wu.astype(f16), w3u=w3u.astype(f16), b3u=b3u.astype(f16),
                  mpat=mpat.astype(f16), binb=bin_, bhb=bh, cbb=cb)
    in_maps = []
    for c in range(N_CORES):
        m = dict(shared)
        m["xT"] = np.ascontiguousarray(xT[:, c * BC:(c + 1) * BC])
        in_maps.append(m)
    return in_maps


def kernel(**inputs):
    from concourse.bass_utils import run_bass_kernel_spmd

    f32r_io = _COMPILED.get("f32r_io", True)
    key = ("nc", f32r_io)
    if key not in _COMPILED:
        _COMPILED[key] = _build(f32r_io)
    nc = _COMPILED[key]
    in_maps = _prep_inputs({k: np.asarray(v) for k, v in inputs.items()})
    res = run_bass_kernel_spmd(nc, in_maps, core_ids=list(range(N_CORES)))
    out = np.concatenate([res.results[c]["yT"] for c in range(N_CORES)], axis=1)
    return np.ascontiguousarray(out.T.astype(np.float32))            # [B, 8]


# revision 5
# speedup vs baseline: 1.0645x; 1.0645x over previous
"""Trainium2 Bass kernel for nn_CompositionalMlp (4-node compositional MLP,
4 experts/node, exact one-hot routing), data-parallel over batch on 8 cores.

Layout: activations kept as [features, batch] ([D,B]) so weights are the
stationary matmul operand and no transposes are needed. All experts are
computed densely; per-row expert selection is done with PE-built broadcast
masks + DVE predicated copies. Matmuls run in float16 (1 cyc/row).
"""
import os
import sys
sys.path.insert(0, "/opt/trn_rl_repo")
os.environ.setdefault("NEURON_RT_RESET_CORES", "1")
import numpy as np

B = 65536
E = 4
F = 32
H = 256
NODES = 4
D_MID = 128
D_OUT = 8
N_CORES = 8
BC = B // N_CORES      # 8192 rows per core
CH = 512               # batch columns per chunk
NCH = BC // CH         # 16 chunks

_COMPILED = {}


def _build(f32r_io: bool):
    import concourse.bass as bass  # noqa: F401
    from concourse import bacc
    import concourse.mybir as mybir
    from concourse.tile import TileContext

    F32 = mybir.dt.float32
    F32R = mybir.dt.float16
    I32 = mybir.dt.int32
    ADD = mybir.AluOpType.add
    MAX = mybir.AluOpType.max
    DT_IN = F32R

    nc = bacc.Bacc("TRN2", target_bir_lowering=False, debug=False,
                   num_devices=N_CORES)
    xT = nc.dram_tensor("xT", [144, BC], DT_IN, kind="ExternalInput").ap()
    win = nc.dram_tensor("win", [16, 32, 256], DT_IN, kind="ExternalInput").ap()
    wa = nc.dram_tensor("wa", [12, 128, 256], DT_IN, kind="ExternalInput").ap()
    wb = nc.dram_tensor("wb", [12, 256, 256], DT_IN, kind="ExternalInput").ap()
    wu = nc.dram_tensor("wu", [12, 256, 128], DT_IN, kind="ExternalInput").ap()
    w3u = nc.dram_tensor("w3u", [4, 256, 8], DT_IN, kind="ExternalInput").ap()
    b3u = nc.dram_tensor("b3u", [4, 8], DT_IN, kind="ExternalInput").ap()
    mpat = nc.dram_tensor("mpat", [3, 4, 128], DT_IN, kind="ExternalInput").ap()
    binb = nc.dram_tensor("binb", [128, 32], F32, kind="ExternalInput").ap()
    bhb = nc.dram_tensor("bhb", [128, 24], F32, kind="ExternalInput").ap()
    cbb = nc.dram_tensor("cbb", [128, 3], F32, kind="ExternalInput").ap()
    yT = nc.dram_tensor("yT", [8, BC], F32, kind="ExternalOutput").ap()

    with TileContext(nc) as tc:
        with (
            tc.tile_pool(name="wload", bufs=1) as wl,
            tc.tile_pool(name="wpool", bufs=1) as wp,
            tc.tile_pool(name="io", bufs=4) as io,
            tc.tile_pool(name="acts", bufs=3) as ap_,
            tc.tile_pool(name="psh", bufs=4, space="PSUM") as psh,
            tc.tile_pool(name="psu", bufs=3, space="PSUM") as psu,
            tc.tile_pool(name="psm", bufs=1, space="PSUM") as psm,
        ):
            def wtile(shape, tag, src):
                t = wp.tile(shape, F32R, tag=tag)
                nc.sync.dma_start(t[:, :], src)
                return t

            # --- resident weights ---
            win_t = [wtile([32, 256], f"win{i}", win[i]) for i in range(16)]
            wa_t = [wtile([128, 256], f"wa{i}", wa[i]) for i in range(12)]
            wb_t = [[wtile([128, 256], f"wb{i}_{k}", wb[i][k * 128:(k + 1) * 128, :])
                     for k in range(2)] for i in range(12)]
            wu_t = [[wtile([128, 128], f"wu{i}_{k}", wu[i][k * 128:(k + 1) * 128, :])
                     for k in range(2)] for i in range(12)]
            w3u_t = [[wtile([128, 8], f"w3u{e}_{k}", w3u[e][k * 128:(k + 1) * 128, :])
                      for k in range(2)] for e in range(4)]
            b3u_t = wtile([4, 8], "b3u", b3u[:, :])
            mp_t = [wtile([4, 128], f"mp{i}", mpat[i]) for i in range(3)]
            bin_t = wp.tile([128, 32], F32, tag="binb")
            nc.sync.dma_start(bin_t[:, :], binb[:, :])
            bh_t = wp.tile([128, 24], F32, tag="bhb")
            nc.sync.dma_start(bh_t[:, :], bhb[:, :])
            cb_t = wp.tile([128, 3], F32, tag="cbb")
            nc.sync.dma_start(cb_t[:, :], cbb[:, :])

            for ci in range(NCH):
                S = slice(ci * CH, (ci + 1) * CH)
                # inputs: per-node feature rows + one-hot rows (base partition 0 each)
                xf = []
                oh = []
                for j in range(NODES):
                    t = io.tile([32, CH], DT_IN, tag=f"xf{j}")
                    nc.sync.dma_start(t[:, :], xT[32 * j:32 * (j + 1), S])
                    xf.append(t)
                    t = io.tile([4, CH], DT_IN, tag=f"oh{j}")
                    nc.sync.dma_start(t[:, :], xT[128 + 4 * j:132 + 4 * j, S])
                    oh.append(t)

                x_prev = None
                for j in range(NODES):
                    # input layer (node0: first layer; nodes 1-3: pre-interface)
                    hin = []
                    for m in range(8):
                        e = m // 2
                        ph = psh.tile([128, CH], F32, tag="ph")
                        nc.tensor.matmul(ph[:, :],
                                         win_t[4 * j + e][:, (m % 2) * 128:(m % 2 + 1) * 128],
                                         xf[j][:, :], start=True, stop=True)
                        ht = ap_.tile([128, CH], F32R, tag=f"a{m}")
                        nc.any.tensor_scalar(ht[:, :], ph[:, :],
                                             bin_t[:, 8 * j + m:8 * j + m + 1], 0.0, ADD, MAX)
                        hin.append(ht)

                    if j == 0:
                        hmid = hin          # node0 has no interface layer
                    else:
                        hmid = []
                        for m in range(8):
                            e = m // 2
                            k2 = m % 2
                            i12 = 4 * (j - 1) + e
                            ph = psh.tile([128, CH], F32, tag="ph")
                            # x_prev-dependent part LAST so the A-parts overlap
                            # with the previous node's combine tail
                            nc.tensor.matmul(ph[:, :],
                                             wb_t[i12][0][:, k2 * 128:(k2 + 1) * 128],
                                             hin[2 * e][:, :], start=True, stop=False)
                            nc.tensor.matmul(ph[:, :],
                                             wb_t[i12][1][:, k2 * 128:(k2 + 1) * 128],
                                             hin[2 * e + 1][:, :], start=False, stop=False)
                            nc.tensor.matmul(ph[:, :],
                                             wa_t[i12][:, k2 * 128:(k2 + 1) * 128],
                                             x_prev[:, :], start=False, stop=True)
                            ht = ap_.tile([128, CH], F32R, tag=f"h{m}")
                            nc.any.tensor_scalar(ht[:, :], ph[:, :],
                                                 bh_t[:, 8 * (j - 1) + m:8 * (j - 1) + m + 1],
                                                 0.0, ADD, MAX)
                            hmid.append(ht)

                    # masks for this node's combine (experts 1..3)
                    pmasks = []
                    for e in (1, 2, 3):
                        mdim = 128 if j < 3 else 8
                        pm = psm.tile([128, CH], F32, tag="mask")
                        nc.tensor.matmul(pm[0:mdim, :], mp_t[e - 1][:, 0:mdim],
                                         oh[j][:, :], start=True, stop=True)
                        pmasks.append(pm)

                    if j < 3:
                        # combine: U_e = W1[e].T @ h_e (+0.1 relu), select expert
                        us = []
                        for e in range(4):
                            pu = psu.tile([128, CH], F32, tag="pu")
                            nc.tensor.matmul(pu[:, :], wu_t[4 * j + e][0][:, :],
                                             hmid[2 * e][:, :], start=True, stop=False)
                            nc.tensor.matmul(pu[:, :], wu_t[4 * j + e][1][:, :],
                                             hmid[2 * e + 1][:, :], start=False, stop=True)
                            ut = ap_.tile([128, CH], F32R, tag=f"u{e}")
                            nc.any.tensor_scalar(ut[:, :], pu[:, :],
                                                 cb_t[:, j:j + 1], 0.0, ADD, MAX)
                            us.append(ut)
                        for e in (1, 2, 3):
                            nc.vector.copy_predicated(us[0][:, :],
                                                      pmasks[e - 1][:, :].bitcast(I32),
                                                      us[e][:, :])
                        x_prev = us[0]
                    else:
                        # output head: U_e = W31[e].T @ h_e + sum_e oh_e b31[e]
                        ys = []
                        for e in range(4):
                            pu = psu.tile([8, CH], F32, tag="pu")
                            nc.tensor.matmul(pu[:, :], w3u_t[e][0][:, :],
                                             hmid[2 * e][:, :], start=True, stop=False)
                            nc.tensor.matmul(pu[:, :], w3u_t[e][1][:, :],
                                             hmid[2 * e + 1][:, :], start=False, stop=False)
                            nc.tensor.matmul(pu[:, :], b3u_t[:, :], oh[3][:, :],
                                             start=False, stop=True)
                            yt_ = ap_.tile([8, CH], F32, tag=f"y{e}")
                            nc.any.tensor_scalar(yt_[:, :], pu[:, :], 0.0, None, ADD)
                            ys.append(yt_)
                        for e in (1, 2, 3):
                            nc.vector.copy_predicated(ys[0][:, :],
                                                      pmasks[e - 1][0:8, :].bitcast(I32),
                                                      ys[e][:, :])
                        nc.sync.dma_start(yT[:, S], ys[0][:, :])
    nc.compile()
    return nc


def _prep_inputs(p):
    f32 = np.float32
    f16 = np.float16
    xT = np.ascontiguousarray(p["input_val"].T.astype(f16))          # [144, B]

    win = np.empty((16, 32, 256), f32)
    bin_ = np.empty((128, 32), f32)
    in_w = [p["W0_0"], p["W1_pre"], p["W2_pre"], p["W3_pre"]]
    in_b = [p["b0_0"], p["b1_pre"], p["b2_pre"], p["b3_pre"]]
    for j in range(4):
        for e in range(4):
            win[4 * j + e] = in_w[j][e]
            for half in range(2):
                bin_[:, 8 * j + 2 * e + half] = in_b[j][e][128 * half:128 * (half + 1)]

    wa = np.empty((12, 128, 256), f32)
    wb = np.empty((12, 256, 256), f32)
    bh = np.empty((128, 24), f32)
    for j in (1, 2, 3):
        w0 = p[f"W{j}_0"]
        b0 = p[f"b{j}_0"]
        for e in range(4):
            i12 = 4 * (j - 1) + e
            wa[i12] = w0[e][0:128, :]
            wb[i12] = w0[e][128:384, :]
            for half in range(2):
                bh[:, 8 * (j - 1) + 2 * e + half] = b0[e][128 * half:128 * (half + 1)]

    wu = np.empty((12, 256, 128), f32)
    cb = np.empty((128, 3), f32)
    for j in (0, 1, 2):
        w1 = p[f"W{j}_1"]
        b1 = p[f"b{j}_1"]
        assert np.ptp(b1, axis=0).max() == 0.0, "combine bias must be expert-constant"
        cb[:, j] = b1[0]
        for e in range(4):
            wu[4 * j + e] = w1[e]

    w3u = np.ascontiguousarray(p["W3_1"].astype(f32))                # [4, 256, 8]
    b3u = np.ascontiguousarray(p["b3_1"].astype(f32))                # [4, 8]
    w3u = w3u  # cast to fp16 in shared dict

    mpat = np.zeros((3, 4, 128), f32)
    for i, e in enumerate((1, 2, 3)):
        mpat[i, e, :] = 1.0

    shared = dict(win=win.astype(f16), wa=wa.astype(f16), wb=wb.astype(f16),
                  wu=wu.astype(f16), w3u=w3u.astype(f16), b3u=b3u.astype(f16),
                  mpat=mpat.astype(f16), binb=bin_, bhb=bh, cbb=cb)
    in_maps = []
    for c in range(N_CORES):
        m = dict(shared)
        m["xT"] = np.ascontiguousarray(xT[:, c * BC:(c + 1) * BC])
        in_maps.append(m)
    return in_maps


def kernel(**inputs):
    from concourse.bass_utils import run_bass_kernel_spmd

    f32r_io = _COMPILED.get("f32r_io", True)
    key = ("nc", f32r_io)
    if key not in _COMPILED:
        _COMPILED[key] = _build(f32r_io)
    nc = _COMPILED[key]
    in_maps = _prep_inputs({k: np.asarray(v) for k, v in inputs.items()})
    res = run_bass_kernel_spmd(nc, in_maps, core_ids=list(range(N_CORES)))
    out = np.concatenate([res.results[c]["yT"] for c in range(N_CORES)], axis=1)
    return np.ascontiguousarray(out.T.astype(np.float32))            # [B, 8]
